# revision 1
# baseline (speedup 1.0000x reference)
"""GroupedEmbedding lookup kernel for 8 Trainium2 NeuronCores.

Sharding: table-wise, 2 tables per core (torchrec-style). Each core holds its
own [2*R, D] weight slab and the matching index slice with table offsets
pre-added; its output is a contiguous [2*L, D] block of the final
[T*L, D] output, so the un-shard is a plain concatenation — no all-to-all.

Device kernel (identical program on all 8 cores, SPMD, raw bass):
  - indices laid out [128, 2048]: partition p owns lookups p*2048 .. p*2048+2047
  - HW indirect DMA consumes ONE index per partition per instruction
    (verified by probing; the simulator's generalized multi-index semantics
    do not hold on hardware). So each gather instruction fetches 128 rows
    (one per partition) into one 64-element column slice of a wide SBUF
    accumulation buffer; after R gathers the [128, R*64] buffer is stored
    with a single HWDGE DMA (contiguous 16R bytes per partition in DRAM).
  - explicit per-slot semaphores (DMA completions are unordered across
    instructions, so cumulative counts on one sem would race).
"""
from contextlib import ExitStack

import numpy as np

import concourse.bass as bass
import concourse.mybir as mybir
from concourse.bass_utils import run_bass_kernel_spmd

# Problem shape (hardcoded per contract)
T = 16          # tables
R = 200000      # rows per table
D = 64          # embedding dim
L = 131072      # lookups per table
NCORES = 8
TPC = T // NCORES   # tables per core

P = 128                     # SBUF partitions
N = TPC * L                 # lookups per core (262144)
M = N // P                  # lookups per partition (2048)
K = 128                     # gathers per chunk (= columns per store)
NBUF = 4                    # dst pipeline slots

_NC_CACHE = {}


def build_nc(K=K, NBUF=NBUF):
    key = (K, NBUF)
    if key in _NC_CACHE:
        return _NC_CACHE[key]
    NCHUNKS = M // K
    nc = bass.Bass("TRN2", target_bir_lowering=False, debug=False)
    idx = nc.dram_tensor("idx", [P, M], mybir.dt.int32, kind="ExternalInput")
    w = nc.dram_tensor("w", [TPC * R, D], mybir.dt.float32, kind="ExternalInput")
    out = nc.dram_tensor("out", [N, D], mybir.dt.float32, kind="ExternalOutput")
    out_v = out.ap().rearrange("(p m) d -> p (m d)", p=P)  # [P, M*D]

    with ExitStack() as ctx:
        idx_tile = ctx.enter_context(
            nc.sbuf_tensor("idx_tile", [P, M], mybir.dt.int32))
        dst = ctx.enter_context(
            nc.sbuf_tensor("dst", [P, NBUF * K * D], mybir.dt.float32))
        idx_sem = ctx.enter_context(nc.semaphore("idx_sem"))
        g_sems = [ctx.enter_context(nc.semaphore(f"g_sem{s}"))
                  for s in range(NBUF)]
        s_sems = [ctx.enter_context(nc.semaphore(f"s_sem{s}"))
                  for s in range(NBUF)]
        block = ctx.enter_context(nc.Block())

        def slot(s):
            return dst[:, s * K * D:(s + 1) * K * D]

        @block.gpsimd
        def _(gpsimd):
            gpsimd.dma_start(idx_tile[:], idx.ap()).then_inc(idx_sem, 16)
            gpsimd.wait_ge(idx_sem, 16)
            for c in range(NCHUNKS):
                s, k = c % NBUF, c // NBUF
                if k > 0:
                    gpsimd.wait_ge(s_sems[s], 16 * k)
                base = s * K * D
                for r in range(K):
                    gpsimd.indirect_dma_start(
                        out=dst[:, base + r * D:base + (r + 1) * D],
                        out_offset=None,
                        in_=w.ap(),
                        in_offset=bass.IndirectOffsetOnAxis(
                            ap=idx_tile[:, c * K + r:c * K + r + 1], axis=0
                        ),
                    ).then_inc(g_sems[s], 16)

        @block.sync
        def _(sync):
            for c in range(NCHUNKS):
                s, k = c % NBUF, c // NBUF
                sync.wait_ge(g_sems[s], 16 * K * (k + 1))
                sync.dma_start(
                    out_v[:, c * K * D:(c + 1) * K * D], slot(s)
                ).then_inc(s_sems[s], 16)

    _NC_CACHE[key] = nc
    return nc


def shard_inputs(indices: np.ndarray, weights: np.ndarray):
    """Full inputs -> per-core in_maps."""
    in_maps = []
    for c in range(NCORES):
        t0 = c * TPC
        w_c = np.ascontiguousarray(weights[t0:t0 + TPC]).reshape(TPC * R, D)
        idx_c = indices[t0:t0 + TPC].astype(np.int64, copy=True)
        idx_c += (np.arange(TPC, dtype=np.int64) * R)[:, None]
        idx_c = idx_c.astype(np.int32).reshape(P, M)
        in_maps.append({"idx": idx_c, "w": w_c})
    return in_maps


def kernel(indices: np.ndarray, weights: np.ndarray, **run_kwargs) -> np.ndarray:
    indices = np.asarray(indices, dtype=np.int32)
    weights = np.asarray(weights, dtype=np.float32)
    assert indices.shape == (T, L) and weights.shape == (T, R, D)

    nc = build_nc()
    in_maps = shard_inputs(indices, weights)
    res = run_bass_kernel_spmd(nc, in_maps, core_ids=list(range(NCORES)),
                               **run_kwargs)
    out = np.concatenate([r["out"] for r in res.results], axis=0)
    kernel.last_results = res
    return out



# revision 2
# speedup vs baseline: 1.9293x; 1.9293x over previous
"""GroupedEmbedding lookup kernel for 8 Trainium2 NeuronCores.

Sharding: table-wise, 2 tables per core (torchrec-style). Each core holds its
own [2*R, D] weight slab and the matching index slice with table offsets
pre-added; its output is a contiguous [2*L, D] block of the final
[T*L, D] output, so the un-shard is a plain concatenation — no all-to-all.

Device kernel (identical program on all 8 cores, SPMD, raw bass):
  - indices laid out [128, 2048]: partition p owns lookups p*2048 .. p*2048+2047
  - HW indirect DMA consumes ONE index per partition per instruction
    (verified by probing; the simulator's generalized multi-index semantics
    do not hold on hardware). So each gather instruction fetches 128 rows
    (one per partition) into one 64-element column slice of a wide SBUF
    accumulation buffer; after R gathers the [128, R*64] buffer is stored
    with a single HWDGE DMA (contiguous 16R bytes per partition in DRAM).
  - explicit per-slot semaphores (DMA completions are unordered across
    instructions, so cumulative counts on one sem would race).
"""
from contextlib import ExitStack

import numpy as np

import concourse.bass as bass
import concourse.mybir as mybir
from concourse.bass_utils import run_bass_kernel_spmd

# Problem shape (hardcoded per contract)
T = 16          # tables
R = 200000      # rows per table
D = 64          # embedding dim
L = 131072      # lookups per table
NCORES = 8
TPC = T // NCORES   # tables per core

P = 128                     # SBUF partitions
N = TPC * L                 # lookups per core (262144)
M = N // P                  # lookups per partition (2048)
K = 128                     # gathers per chunk (= columns per store)
NBUF = 4                    # dst pipeline slots

_NC_CACHE = {}


def build_nc(K=K, NBUF=NBUF):
    key = (K, NBUF)
    if key in _NC_CACHE:
        return _NC_CACHE[key]
    NCHUNKS = M // K
    # 32KB/partition SWDGE descriptor carveout (default 16KB): the 9-desc
    # ring footprint per indirect DMA fills the default 1024-desc ring every
    # ~113 instructions, costing a ~3.7us reclaim stall per chunk.
    nc = bass.Bass("TRN2", target_bir_lowering=False, debug=False,
                   dynamic_dma_scratch_size=32768)
    idx = nc.dram_tensor("idx", [P, M], mybir.dt.int32, kind="ExternalInput")
    w = nc.dram_tensor("w", [TPC * R, D], mybir.dt.float32, kind="ExternalInput")
    out = nc.dram_tensor("out", [N, D], mybir.dt.float32, kind="ExternalOutput")
    out_v = out.ap().rearrange("(p m) d -> p (m d)", p=P)  # [P, M*D]

    with ExitStack() as ctx:
        idx_tile = ctx.enter_context(
            nc.sbuf_tensor("idx_tile", [P, M], mybir.dt.int32))
        dst = ctx.enter_context(
            nc.sbuf_tensor("dst", [P, NBUF * K * D], mybir.dt.float32))
        idx_sem = ctx.enter_context(nc.semaphore("idx_sem"))
        g_sems = [ctx.enter_context(nc.semaphore(f"g_sem{s}"))
                  for s in range(NBUF)]
        s_sems = [ctx.enter_context(nc.semaphore(f"s_sem{s}"))
                  for s in range(NBUF)]
        block = ctx.enter_context(nc.Block())

        def slot(s):
            return dst[:, s * K * D:(s + 1) * K * D]

        @block.gpsimd
        def _(gpsimd):
            gpsimd.dma_start(idx_tile[:], idx.ap()).then_inc(idx_sem, 16)
            gpsimd.wait_ge(idx_sem, 16)
            for c in range(NCHUNKS):
                s, k = c % NBUF, c // NBUF
                if k > 0:
                    gpsimd.wait_ge(s_sems[s], 16 * k)
                base = s * K * D
                for r in range(K):
                    gpsimd.indirect_dma_start(
                        out=dst[:, base + r * D:base + (r + 1) * D],
                        out_offset=None,
                        in_=w.ap(),
                        in_offset=bass.IndirectOffsetOnAxis(
                            ap=idx_tile[:, c * K + r:c * K + r + 1], axis=0
                        ),
                    ).then_inc(g_sems[s], 16)

        @block.sync
        def _(sync):
            for c in range(NCHUNKS):
                s, k = c % NBUF, c // NBUF
                sync.wait_ge(g_sems[s], 16 * K * (k + 1))
                sync.dma_start(
                    out_v[:, c * K * D:(c + 1) * K * D], slot(s)
                ).then_inc(s_sems[s], 16)

    _NC_CACHE[key] = nc
    return nc


def shard_inputs(indices: np.ndarray, weights: np.ndarray):
    """Full inputs -> per-core in_maps."""
    in_maps = []
    for c in range(NCORES):
        t0 = c * TPC
        w_c = np.ascontiguousarray(weights[t0:t0 + TPC]).reshape(TPC * R, D)
        idx_c = indices[t0:t0 + TPC].astype(np.int64, copy=True)
        idx_c += (np.arange(TPC, dtype=np.int64) * R)[:, None]
        idx_c = idx_c.astype(np.int32).reshape(P, M)
        in_maps.append({"idx": idx_c, "w": w_c})
    return in_maps


def kernel(indices: np.ndarray, weights: np.ndarray, **run_kwargs) -> np.ndarray:
    indices = np.asarray(indices, dtype=np.int32)
    weights = np.asarray(weights, dtype=np.float32)
    assert indices.shape == (T, L) and weights.shape == (T, R, D)

    nc = build_nc()
    in_maps = shard_inputs(indices, weights)
    res = run_bass_kernel_spmd(nc, in_maps, core_ids=list(range(NCORES)),
                               **run_kwargs)
    out = np.concatenate([r["out"] for r in res.results], axis=0)
    kernel.last_results = res
    return out

